# revision 26
# baseline (speedup 1.0000x reference)
"""Trainium2 Bass kernel for CustomPositionsPiecewiseConv2d.

Math: for knots positions=[-1,-.5,0,.5,1] and x in [0,1], only the last two
spline intervals are active.  With g2 = 2*min(v,0.5) and c4 = relu(2v-1)
the per-tap contribution is
    contrib = W2 + g2*(W3-W2) + c4*(W4-W3)
so  out = G2 (x) (W3-W2) + C4 (x) (W4-W3) + bias + sum_ck W2.

The coefficient planes (g2, c4) are elementwise maps of x and are built on
the host directly in the matmul-ready Y layout (the device kernel keeps all
conv FLOPs).  Y packs the contraction K axis: partition 2c+g holds plane
(channel c, map g) at row offset 0, partition 64+2c+g the same at row
offset +1, so one K=128 matmul at column offset kw contracts taps (0,kw)
and (1,kw) simultaneously.  The kh=2 row either reuses Y at rows+2 with a
zero-weighted upper half (6 steps/tile) or, in five-step mode, reads a
second buffer Y2 whose upper half is column-shifted by 1 so taps (2,0) and
(2,1) pair as well (5 steps/tile, 17% less PE work, ~2.2MB more DMA).

Output is written [O, IPC, H, W] (per-partition-contiguous 2KB segments)
and transposed on host.  Sharding: data-parallel, 2 images per core.

Schedule notes: a short warm matmul batch starts the HAM activity ramp
(PE runs at half rate until the grant, ~4.2us after it first goes busy);
trailing dummy matmuls keep the grant alive through the drain/store tail
so the fixed semaphore-reset epilogue (~250 sems) runs at full clock.
"""

import numpy as np

B, C, H, W = 16, 32, 64, 64
O, P, KH, KW = 128, 5, 3, 3
NCORES = 8
IPC = B // NCORES            # images per core
HP, WP = H + 2, W + 2        # padded plane (pad=1)
RT = 8                       # output rows per L-tile (PSUM: 1 bank/tile)
NT = H // RT                 # L-tiles per image
ATOL = 1e-5
RTOL = 1e-5

FIVE_STEP = True             # 5 matmuls/tile (needs Y2) vs 6 (Y only)
N_WARM = 5                   # leading dummy matmuls (HAM activity ramp)
N_TRAIL = 0                  # trailing dummies (epilogue is clock-insensitive)

# Y row chunks per image: small first chunk for an early first matmul
YCH = (
    [(0, 8), (8, 24), (24, 40), (40, 56), (56, 64)]
    if FIVE_STEP
    else [(0, 8), (8, 24), (24, 40), (40, 56), (56, 66)]
)
Y2CH = [(0, 8), (8, 24), (24, 40), (40, 56), (56, 64)]


# ---------------------------------------------------------------- host math


def _isclose_np(a, b):
    return np.abs(a - b) <= np.float32(ATOL) + np.float32(RTOL) * np.abs(b)


def _reference_np(x, weights, bias, positions):
    """Direct numpy port of the reference (fallback path)."""
    EPS = 1e-6
    Bn, Cn, Hn, Wn = x.shape
    On, _, Pn, KHn, KWn = weights.shape
    xp = np.pad(x, ((0, 0), (0, 0), (1, 1), (1, 1)))
    cols = [
        xp[:, :, i : i + Hn, j : j + Wn] for i in range(KHn) for j in range(KWn)
    ]
    pat = np.stack(cols, axis=2)
    v = pat.reshape(Bn, Cn, KHn * KWn, Hn * Wn).astype(np.float32)

    left, right = positions[:-1], positions[1:]
    denom = right - left
    denom = np.where(denom == 0, np.float32(EPS), denom)
    varc = (1.0 / denom).astype(np.float32)
    const = (-left * varc).astype(np.float32)

    m_first = _isclose_np(v, positions[0])
    m_last = _isclose_np(v, positions[-1])
    in_range = (~(m_first | m_last)) & (v >= positions[0]) & (v <= positions[-1])

    coeff = np.zeros(v.shape + (Pn,), np.float32)
    coeff[..., 0] += m_first.astype(np.float32)
    coeff[..., Pn - 1] += m_last.astype(np.float32)
    for p in range(Pn - 1):
        m = (in_range & (v >= positions[p]) & (v < positions[p + 1])).astype(
            np.float32
        )
        t = v * varc[p] + const[p]
        coeff[..., p] += m * (1.0 - t)
        coeff[..., p + 1] += m * t

    Wk = np.transpose(weights, (0, 1, 3, 4, 2)).reshape(On, Cn, KHn * KWn, Pn)
    ident = np.all(np.abs(Wk - 1.0) <= np.float32(ATOL + RTOL), axis=-1)
    Wk_eff = np.where(ident[..., None], np.float32(0.0), Wk)

    out = np.einsum("bcklp,ockp->bol", coeff, Wk_eff, optimize=True)
    out = out + np.einsum(
        "bckl,ock->bol", v, ident.astype(np.float32), optimize=True
    )
    out = out + bias[None, :, None]
    return out.reshape(Bn, On, Hn, Wn).astype(np.float32)


def _host_weights(weights, bias):
    """Fold the spline into per-step stationary weights.

    Returns (wstat [128, NS, O] bf16, bias_eff [O] f32, ident_any).
    K-lane layout matches Y: lane 2c+g (g=0: g2 -> W3-W2, g=1: c4 -> W4-W3).
    Steps 0-2 (kw=s): lower lanes tap (0,kw), upper tap (1,kw).
    Six-step: steps 3-5 (kw=s-3): lower tap (2,kw), upper zero.
    Five-step: step 3: lower tap (2,0), upper tap (2,1); step 4: lower
    tap (2,2), upper zero.
    """
    import ml_dtypes

    ns = 5 if FIVE_STEP else 6
    Wk = np.transpose(weights, (0, 1, 3, 4, 2)).reshape(O, C, KH * KW, P)
    ident = np.all(np.abs(Wk - 1.0) <= np.float32(ATOL + RTOL), axis=-1)
    ident_any = bool(ident.any())
    W2 = Wk[:, :, :, 2].astype(np.float64)
    W3 = Wk[:, :, :, 3].astype(np.float64)
    W4 = Wk[:, :, :, 4].astype(np.float64)
    G = W3 - W2                       # multiplies g2   [O, C, K2]
    Q = W4 - W3                       # multiplies c4

    wstat = np.zeros((128, ns, O), np.float64)

    def put(lane0, s, kh, kw):
        wstat[lane0 + 0 : lane0 + 64 : 2, s] = G[:, :, kh * KW + kw].T
        wstat[lane0 + 1 : lane0 + 64 : 2, s] = Q[:, :, kh * KW + kw].T

    for kw in range(KW):
        put(0, kw, 0, kw)
        put(64, kw, 1, kw)
    if FIVE_STEP:
        put(0, 3, 2, 0)
        put(64, 3, 2, 1)
        put(0, 4, 2, 2)
    else:
        for kw in range(KW):
            put(0, KW + kw, 2, kw)

    bias_eff = (bias.astype(np.float64) + W2.sum(axis=(1, 2))).astype(np.float32)
    return (
        np.ascontiguousarray(wstat.astype(ml_dtypes.bfloat16)),
        bias_eff,
        ident_any,
    )


def _host_coeff(x):
    """Build the matmul-ready coefficient buffers on the host.

    Returns (Y [B, 128, NY, WP] bf16, Y2 [B, 128, H, WP] bf16 or None).
    plane[c, g] is the zero-padded g2/c4 map; Y lane 2c+g = plane rows r,
    lane 64+2c+g = plane rows r+1.  Six-step keeps all HP rows (steps 3-5
    read rows r+2 on the lower half).  Five-step trims Y to 64 rows and
    adds Y2: lower = plane rows 2:66, upper = same but columns +1.
    """
    import ml_dtypes

    ny = H if FIVE_STEP else HP
    plane = np.zeros((B, C, 2, HP + 1, WP), np.float32)  # +1 pad row for r+1
    plane[:, :, 0, 1 : H + 1, 1 : W + 1] = 2.0 * np.minimum(x, 0.5)
    plane[:, :, 1, 1 : H + 1, 1 : W + 1] = np.maximum(2.0 * x - 1.0, 0.0)
    lanes = plane.reshape(B, 64, HP + 1, WP)  # lane = 2c+g

    Y = np.empty((B, 128, ny, WP), np.float32)
    Y[:, 0:64] = lanes[:, :, 0:ny]
    Y[:, 64:128] = lanes[:, :, 1 : ny + 1]
    Y2 = None
    if FIVE_STEP:
        Y2 = np.empty((B, 128, H, WP), np.float32)
        Y2[:, 0:64] = lanes[:, :, 2 : 2 + H]
        Y2[:, 64:128, :, 0 : WP - 1] = lanes[:, :, 2 : 2 + H, 1:WP]
        Y2[:, 64:128, :, WP - 1] = 0.0
        Y2 = np.ascontiguousarray(Y2.astype(ml_dtypes.bfloat16))
    return np.ascontiguousarray(Y.astype(ml_dtypes.bfloat16)), Y2


# ---------------------------------------------------------------- device IR


def _build_nc():
    import concourse.tile as tile
    from concourse import bacc, mybir

    f32 = mybir.dt.float32
    bf16 = mybir.dt.bfloat16
    Alu = mybir.AluOpType
    Act = mybir.ActivationFunctionType

    ns = 5 if FIVE_STEP else 6
    ny = H if FIVE_STEP else HP

    nc = bacc.Bacc("TRN2", target_bir_lowering=False, debug=False,
                   num_devices=NCORES)
    y_d = nc.dram_tensor("y", [IPC, 128, ny, WP], bf16, kind="ExternalInput").ap()
    y2_d = None
    if FIVE_STEP:
        y2_d = nc.dram_tensor(
            "y2", [IPC, 128, H, WP], bf16, kind="ExternalInput"
        ).ap()
    w_d = nc.dram_tensor("wstat", [128, ns, O], bf16, kind="ExternalInput").ap()
    b_d = nc.dram_tensor("bias", [O, 1], f32, kind="ExternalInput").ap()
    o_d = nc.dram_tensor("out", [O, IPC, H, W], bf16, kind="ExternalOutput").ap()

    with tile.TileContext(nc) as tc:
        with (
            tc.tile_pool(name="const", bufs=1) as constp,
            tc.tile_pool(name="ybuf", bufs=1) as ybufp,
            tc.tile_pool(name="psum", bufs=1, space="PSUM") as psump,
            tc.tile_pool(name="osb", bufs=3) as osbp,
        ):
            # ---- w16 leads the sync ring (first matmul gates on it), Y
            # chunks ride right behind; img0's Y2 leads the Act ring.  The
            # two HW-DGE rings generate descriptors in parallel; triggers
            # on one ring serialize at ~600ns each. ----
            w16 = constp.tile([128, ns, O], bf16)
            nc.sync.dma_start(w16[:], w_d[:])
            b_sb = constp.tile([O, 1], f32)

            # PE warmup operand; first warm matmul gates on this memset
            zb = constp.tile([128, 512], bf16)
            nc.gpsimd.memset(zb[:], 0.0)

            # warm the scalar ACT table off the critical path (drains use
            # Identity-with-bias)
            tiny = constp.tile([C, 1], f32)
            nc.gpsimd.memset(tiny[:], 0.0)
            nc.scalar.activation(
                tiny[:], tiny[:], Act.Identity, bias=tiny[:, 0:1], scale=1.0
            )

            warm_ctr = [0]

            def warm(nmm):
                """Dummy matmuls (results never read) to keep the PE busy."""
                w = warm_ctr[0]
                warm_ctr[0] += 1
                pw = psump.tile(
                    [O, RT * W], f32, name=f"ps_warm{w}", tag=f"ps{w % 2}"
                )
                for _ in range(nmm):
                    nc.tensor.matmul(
                        pw[:, 0:512], zb[:, 0:128], zb[:], start=True, stop=True
                    )

            # ---- coefficient loads, interleaved with the matmul stream ----
            Ys, Y2s = [], []
            for i in range(IPC):
                Ys.append(ybufp.tile([128, ny, WP], bf16, name=f"Y{i}"))
                if FIVE_STEP:
                    Y2s.append(ybufp.tile([128, H, WP], bf16, name=f"Y2{i}"))

            def load_img(i, eng, eng2):
                for (r0, r1), (q0, q1) in zip(YCH, Y2CH):
                    eng.dma_start(Ys[i][:, r0:r1, :], y_d[i, :, r0:r1, :])
                    if FIVE_STEP:
                        eng2.dma_start(
                            Y2s[i][:, q0:q1, :], y2_d[i, :, q0:q1, :]
                        )

            # both rings feed img0 in parallel: full-rate tiles consume a
            # Y+Y2 chunk pair per ~1.1us, one ring only supplies one per
            # ~1.2us.  img1 loads follow, swapped across rings; bias rides
            # after img0 (first drain needs it ~3us later).
            load_img(0, nc.sync, nc.scalar)
            nc.scalar.dma_start(b_sb[:], b_d[:])
            warm(N_WARM)
            load_img(1, nc.scalar, nc.sync)

            def mms_for_tile(i, t, ps):
                Y = Ys[i]
                for s in range(ns):
                    if s < KW:
                        rhs = Y[:, t * RT : t * RT + RT, s : s + W]
                    elif FIVE_STEP:
                        kw = 0 if s == 3 else 2
                        rhs = Y2s[i][:, t * RT : t * RT + RT, kw : kw + W]
                    else:
                        kw = s - KW
                        rhs = Y[:, t * RT + 2 : t * RT + 2 + RT, kw : kw + W]
                    nc.tensor.matmul(
                        ps[:], w16[:, s, :], rhs,
                        start=(s == 0), stop=(s == ns - 1),
                    )

            HB = RT * W // 2
            for i in range(IPC):
                pss = [
                    psump.tile([O, RT * W], f32, name=f"ps{t}", tag=f"ps{t}")
                    for t in range(NT)
                ]
                last_img = i == IPC - 1
                # pair-outer: finish one PSUM bank pair, then drain and
                # store it while the next pair accumulates
                for tp in range(NT // 2):
                    t0, t1 = 2 * tp, 2 * tp + 1
                    if last_img and tp == NT // 2 - 1:
                        # tile-serial final pair: t0's drain+store stream
                        # during t1's matmuls, and t1 drains/stores in
                        # parallel halves so the kernel end isn't gated on
                        # one long drain + one long store transfer
                        mms_for_tile(i, t0, pss[t0])
                        osb = osbp.tile([O, 2 * RT * W], bf16, name="osb")
                        nc.scalar.activation(
                            osb[:, 0 : RT * W], pss[t0][:], Act.Identity,
                            bias=b_sb[:, 0:1], scale=1.0,
                        )
                        nc.scalar.dma_start(
                            o_d[:, i, t0 * RT : t0 * RT + RT, :],
                            osb[:, 0 : RT * W]
                            .rearrange("o (r w) -> o r w", r=RT),
                        )
                        mms_for_tile(i, t1, pss[t1])
                        ob1 = osb[:, RT * W : 2 * RT * W]
                        nc.scalar.activation(
                            ob1[:, 0:HB], pss[t1][:, 0:HB], Act.Identity,
                            bias=b_sb[:, 0:1], scale=1.0,
                        )
                        nc.vector.tensor_scalar(
                            ob1[:, HB : RT * W], pss[t1][:, HB : RT * W],
                            b_sb[:, 0:1], None, Alu.add,
                        )
                        for eng, h in ((nc.scalar, 0), (nc.sync, 1)):
                            eng.dma_start(
                                o_d[
                                    :, i,
                                    t1 * RT + h * RT // 2
                                    : t1 * RT + (h + 1) * RT // 2,
                                    :,
                                ],
                                ob1[:, h * HB : (h + 1) * HB]
                                .rearrange("o (r w) -> o r w", r=RT // 2),
                            )
                        continue
                    for t in (t0, t1):
                        mms_for_tile(i, t, pss[t])
                    osb = osbp.tile([O, 2 * RT * W], bf16, name="osb")
                    nc.scalar.activation(
                        osb[:, 0 : RT * W], pss[t0][:], Act.Identity,
                        bias=b_sb[:, 0:1], scale=1.0,
                    )
                    nc.vector.tensor_scalar(
                        osb[:, RT * W : 2 * RT * W], pss[t1][:],
                        b_sb[:, 0:1], None, Alu.add,
                    )
                    nc.scalar.dma_start(
                        o_d[:, i, 2 * tp * RT : 2 * tp * RT + 2 * RT, :],
                        osb[:].rearrange("o (r w) -> o r w", r=2 * RT),
                    )

            if N_TRAIL:
                warm(N_TRAIL)
    nc.compile()
    return nc


# ---------------------------------------------------------------- entry


def _prep(inputs):
    x = np.ascontiguousarray(np.asarray(inputs["x"], dtype=np.float32))
    weights = np.ascontiguousarray(np.asarray(inputs["weights"], dtype=np.float32))
    bias = np.ascontiguousarray(np.asarray(inputs["bias"], dtype=np.float32))
    positions = np.ascontiguousarray(
        np.asarray(inputs["positions"], dtype=np.float32)
    )
    return x, weights, bias, positions


def _fast_path_ok(x, positions):
    expect = np.linspace(-1.0, 1.0, P, dtype=np.float32)
    return (
        x.shape == (B, C, H, W)
        and positions.shape == (P,)
        and np.array_equal(positions, expect)
        and float(x.min()) >= 0.0
        and float(x.max()) <= 1.0
    )


def kernel(**inputs):
    x, weights, bias, positions = _prep(inputs)
    if not _fast_path_ok(x, positions):
        return _reference_np(x, weights, bias, positions)

    wstat, bias_eff, ident_any = _host_weights(weights, bias)
    if ident_any:
        # identity-shortcut weights present: needs the raw-v plane; use the
        # exact fallback rather than a rarely-exercised device path
        return _reference_np(x, weights, bias, positions)

    from concourse.bass_utils import run_bass_kernel_spmd

    Yh, Y2h = _host_coeff(x)
    nc = _build_nc()
    bias2d = np.ascontiguousarray(bias_eff.reshape(O, 1))
    in_maps = []
    for i in range(NCORES):
        m = {
            "y": Yh[i * IPC : (i + 1) * IPC],
            "wstat": wstat,
            "bias": bias2d,
        }
        if FIVE_STEP:
            m["y2"] = Y2h[i * IPC : (i + 1) * IPC]
        in_maps.append(m)
    res = run_bass_kernel_spmd(nc, in_maps, core_ids=list(range(NCORES)))
    out = np.concatenate(
        [
            np.asarray(res.results[i]["out"])
            .astype(np.float32)
            .transpose(1, 0, 2, 3)
            for i in range(NCORES)
        ],
        axis=0,
    )
    return np.ascontiguousarray(out)


# ------------------------------------------------------------ dev utilities


def _run_sim(inputs):
    """CoreSim single-core run (images 0..IPC-1) for correctness debugging."""
    from concourse.bass_interp import CoreSim

    x, weights, bias, positions = _prep(inputs)
    assert _fast_path_ok(x, positions)
    wstat, bias_eff, ident_any = _host_weights(weights, bias)
    assert not ident_any

    Yh, Y2h = _host_coeff(x)
    nc = _build_nc()
    sim = CoreSim(nc)
    sim.tensor("y")[:] = Yh[:IPC]
    if FIVE_STEP:
        sim.tensor("y2")[:] = Y2h[:IPC]
    sim.tensor("wstat")[:] = wstat
    sim.tensor("bias")[:] = bias_eff.reshape(O, 1)
    sim.simulate()
    return (
        np.array(sim.tensor("out")).astype(np.float32).transpose(1, 0, 2, 3)
    )
